# revision 73
# baseline (speedup 1.0000x reference)
"""Trainium2 Bass kernel for LayerNorm + multi-head attention (B=4, S=2048,
D=1024, H=16) with sigmoid(s-mu)*exp(s) row-normalized attention weights.

Sharding: 8 cores = 4 batches x 2 head-groups (8 heads each). Each core
computes LN + its head-group's q/k/v projections + attention + a partial
output projection; the host sums the two partials per batch and adds bo.

v2 design (vs v1 baseline at ~695us):
  * every matmul is bf16 (v1 used fp32r HIGH, which is bf16-precision anyway)
  * phase B emits one Exp + one custom-DVE op per [128,1024] PSUM bank-pair
    (both heads of a pair in one instruction) to amortize fixed overheads
  * AV carries the denominator as a rider row: par-e v-block [64v | 1] puts
    den at psum row 64; par-o v-block [63 zeros | 1 | 64v] puts den at row 63
    and values at rows 64..127, so the normalize multiply needs no partition
    shifts anywhere
  * normalization: Scalar copies den rows out of PSUM, DMA shifts them to
    partition 0/1, one custom-DVE reciprocal, GpSimd partition_broadcast to
    [128,512], DVE multiply -> attn (bf16)
  * phase C (output projection) is interleaved at each 512-token boundary,
    accumulating in a PSUM tile borrowed from the scores pool
  * software-pipelined emission: per block period the PE queue gets
    [sc(i,t), av(i-1,t)] x16 interleaved, Scalar gets exp(i,*) + den copies,
    Vector gets custom(i,*) with recip/mults of block i-1 slotted mid-stream
"""

import sys

if "/opt/trn_rl_repo" not in sys.path:
    sys.path.insert(0, "/opt/trn_rl_repo")

import numpy as np
import ml_dtypes as _ml

D_MODEL = 1024
N_HEADS = 16
HEAD_DIM = 64
SEQ = 2048
BATCH = 4
N_CORES = 8
EPS = 1e-6
SCALE = float(HEAD_DIM) ** 0.5  # 8.0

# Per-core partitioning
H_LOC = 8                 # heads per core
W_LOC = H_LOC * HEAD_DIM  # 512 local projection width
N_DT = D_MODEL // 128     # 8 d-tiles
N_TT = SEQ // 128         # 16 token tiles
N_TC = 4                  # token chunks of 512
N_HP = 4                  # head pairs per core
N_T1B = 4                 # 512-wide t1 blocks
# v_aug pair-block: [64 v_e | 1 one] then [zeros | one@32 | zeros | 64 v_o]
# (o-rider at col 32 so the den row lands on a 32-aligned PSUM partition)
PB_E_OFF = 0      # v_e at cols 0..64, one at col 64
PB_O_OFF = 65     # o-block: one at col 32, v_o at cols 64..128
PB_W = 65 + 128   # 193 cols per head pair

# Custom-DVE fused reciprocal-of-act2 constants (seed interval tuned;
# ~0.4% max rel err), for g = 1/(y + y^2).
RA_C0 = -0.234
RA_C1 = 2.0
_DEBUG = False


def _get_recip_act2_op():
    """Register (once) and return the custom DVE op: out = 1/(x + x^2)."""
    import concourse.dve_ops as dve_ops

    if hasattr(dve_ops, "RECIP_ACT2_ANT"):
        return dve_ops.RECIP_ACT2_ANT

    from concourse.dve_spec import Spec, Src0, C0, C1, Bin, AluOp, sq, lower, _has_src1
    from concourse.dve_uop import DveOpSpec

    _w = sq(Src0) + Src0
    _nw = Bin(AluOp.BITWISE_NOT, _w, _w)
    _y0 = _nw * C0
    _body = _y0 * (C1 - _w * _y0)

    def _ref(in0, in1, s0, s1, imm2):
        x = np.asarray(in0).astype(np.float32)
        w = (x + x * x).astype(np.float32)
        nw = (~w.view(np.int32)).view(np.float32)
        if isinstance(s0, np.ndarray):
            s0 = s0.astype(np.float32)
        if isinstance(s1, np.ndarray):
            s1 = s1.astype(np.float32)
        y0 = (nw * np.float32(s0) if not isinstance(s0, np.ndarray) else nw * s0).astype(np.float32)
        c1 = np.float32(s1) if not isinstance(s1, np.ndarray) else s1
        return (y0 * (c1 - w * y0)).astype(np.float32)

    spec = Spec(body=_body, reference=_ref)
    name = "RECIP_ACT2_ANT"
    row = max(dve_ops._SUB_OPCODE_FOR_NAME.values()) + 1
    assert row < 0x20
    dve_ops._SUB_OPCODE_FOR_NAME[name] = row
    shas = {}
    for ver in ("v3", "v4"):
        compiled = DveOpSpec(
            name=name, opcode=row, uops=lower(spec, ver=ver), rd1_en=_has_src1(spec)
        )
        shas[ver] = compiled.sha(ver)
    op = dve_ops.DveOp(name, spec, subdim=False, uops_sha=shas)
    dve_ops.OPS.append(op)
    dve_ops.CUSTOM_DVE_SPECS[name] = spec
    dve_ops.RECIP_ACT2_ANT = op
    return op


def _broadcast_ap(ap, parts):
    """Partition-broadcast a 1-D AP of shape [N] to [parts, N] (step-0)."""
    import concourse.bass as bass

    steps = [list(p) for p in ap.ap]
    return bass.AP(tensor=ap.tensor, offset=ap.offset, ap=[[0, parts]] + steps)


def _bcast_rows(ap2d, parts):
    """Partition-broadcast a [1, N] AP to [parts, N] (step-0 partition dim)."""
    import concourse.bass as bass

    inner = [list(p) for p in ap2d.ap[1:]]
    return bass.AP(tensor=ap2d.tensor, offset=ap2d.offset, ap=[[0, parts]] + inner)


def _build_program(mu_val: float):
    import concourse.bass as bass
    import concourse.mybir as mybir
    import concourse.tile as tile
    from concourse import bacc
    from concourse.dve_ops import RECIPROCAL_APPROX_FAST, RECIP_APPROX_FAST_CONSTS

    recip_act2 = _get_recip_act2_op()

    f32 = mybir.dt.float32
    bf16 = mybir.dt.bfloat16
    AF = mybir.ActivationFunctionType
    ALU = mybir.AluOpType
    rc = RECIP_APPROX_FAST_CONSTS

    nc = bacc.Bacc("TRN2", target_bir_lowering=False, debug=False,
                   num_devices=N_CORES)

    x_d = nc.dram_tensor("x", [SEQ, D_MODEL], f32, kind="ExternalInput").ap()
    wq_d = nc.dram_tensor("wqT", [D_MODEL, W_LOC], bf16, kind="ExternalInput").ap()
    wk_d = nc.dram_tensor("wkT", [D_MODEL, W_LOC], bf16, kind="ExternalInput").ap()
    wv_d = nc.dram_tensor("wvT", [D_MODEL, W_LOC], bf16, kind="ExternalInput").ap()
    wo_d = nc.dram_tensor("woT", [W_LOC, D_MODEL], bf16, kind="ExternalInput").ap()
    bq_d = nc.dram_tensor("bq", [W_LOC], f32, kind="ExternalInput").ap()
    bk_d = nc.dram_tensor("bk", [W_LOC], f32, kind="ExternalInput").ap()
    bv_d = nc.dram_tensor("bv", [1, W_LOC], bf16, kind="ExternalInput").ap()
    out_d = nc.dram_tensor("out", [SEQ, D_MODEL], f32, kind="ExternalOutput").ap()
    # per-block scratch rows for the reciprocal partition-broadcast bounce
    r_d = nc.dram_tensor("rscratch", [N_T1B * N_HP, 1024], bf16, kind="Internal").ap()
    dbg = {}
    if _DEBUG:
        dbg["qT"] = nc.dram_tensor("dbg_qT", [128, N_HP, SEQ], bf16, kind="ExternalOutput").ap()
        dbg["kT"] = nc.dram_tensor("dbg_kT", [128, N_HP, SEQ], bf16, kind="ExternalOutput").ap()
        dbg["v"] = nc.dram_tensor("dbg_v", [128, N_TT, N_HP * PB_W], bf16, kind="ExternalOutput").ap()
        dbg["attn"] = nc.dram_tensor("dbg_attn", [128, N_HP, SEQ], bf16, kind="ExternalOutput").ap()

    with tile.TileContext(nc) as tc:
        with (
            tc.tile_pool(name="consts", bufs=1) as consts,
            tc.tile_pool(name="qkv", bufs=1) as qkv_pool,
        ):
            eps_sb = consts.tile([128, 1], f32)
            nc.vector.memset(eps_sb, EPS)
            bq_sb = consts.tile([128, 4], f32)
            nc.sync.dma_start(out=bq_sb, in_=bq_d.rearrange("(a p) -> p a", p=128))
            bk_sb = consts.tile([128, 4], f32)
            nc.sync.dma_start(out=bk_sb, in_=bk_d.rearrange("(a p) -> p a", p=128))
            bv_row = consts.tile([1, W_LOC], bf16)
            nc.sync.dma_start(out=bv_row, in_=bv_d)
            ones_row = consts.tile([1, 128], bf16)
            nc.vector.memset(ones_row, 1.0)

            # persistent activations (bf16)
            qT = qkv_pool.tile([128, N_HP, SEQ], bf16)
            kT = qkv_pool.tile([128, N_HP, SEQ], bf16)
            xT = qkv_pool.tile([128, N_DT, SEQ], bf16)
            wq_sb = qkv_pool.tile([128, N_DT, W_LOC], bf16)
            nc.sync.dma_start(out=wq_sb,
                              in_=wq_d.rearrange("(a p) j -> p a j", p=128))
            v_aug = qkv_pool.tile([128, N_TT, N_HP * PB_W], bf16)
            nc.gpsimd.memset(v_aug, 0.0)
            va_pair = v_aug.rearrange("p t (h c) -> p t h c", c=PB_W)
            # rider ones: col 64 (par-e) and col PB_O_OFF+32 (par-o)
            nc.vector.memset(va_pair[:, :, :, 64:65], 1.0)
            nc.vector.memset(va_pair[:, :, :, PB_O_OFF + 32:PB_O_OFF + 33], 1.0)

            _phase_a(tc, nc, mybir, x_d, wq_d, wq_sb, wk_d, wv_d, bq_sb,
                     bk_sb, bv_row, ones_row, eps_sb, qT, kT, xT, v_aug,
                     va_pair)

            with tc.tile_pool(name="attn", bufs=1) as attn_pool:
                attn_sb = attn_pool.tile([128, N_HP, SEQ], bf16)
                _phase_bc(tc, nc, mybir, qT, kT, xT, wq_sb, bq_sb, v_aug,
                          va_pair, attn_sb, wo_d, out_d, r_d, mu_val,
                          recip_act2, RECIPROCAL_APPROX_FAST, rc)
                if _DEBUG:
                    nc.sync.dma_start(out=dbg["qT"], in_=qT)
                    nc.sync.dma_start(out=dbg["kT"], in_=kT)
                    nc.sync.dma_start(out=dbg["v"], in_=v_aug)
                    nc.sync.dma_start(out=dbg["attn"], in_=attn_sb)

    nc.compile()
    return nc


def _phase_a(tc, nc, mybir, x_d, wq_d, wq_sb, wk_d, wv_d, bq_sb, bk_sb,
             bv_row, ones_row, eps_sb, qT, kT, xT, v_aug, va_pair):
    """LN + DMA-transpose + k/v projections (+ q chunk 0), all-bf16 matmuls.

    xT is built by XBAR DMA-transpose (no PE transposes, no PSUM drains).
    q chunks 1..3 are emitted inside phase B to shorten the serial window.
    """
    f32 = mybir.dt.float32
    bf16 = mybir.dt.bfloat16
    AF = mybir.ActivationFunctionType
    ALU = mybir.AluOpType

    with (
        tc.tile_pool(name="wkv", bufs=1) as wp,
        tc.tile_pool(name="ph_x", bufs=4) as xp,
        tc.tile_pool(name="ph_s", bufs=6) as sp,
        tc.tile_pool(name="psA", bufs=3, space="PSUM") as psA,
    ):
        w_sbs = {}
        for wname, w_d in (("k", wk_d), ("v", wv_d)):
            w_sbs[wname] = wp.tile([128, N_DT, W_LOC], bf16,
                                   name=f"w{wname}", tag=f"w{wname}")
            nc.sync.dma_start(
                out=w_sbs[wname],
                in_=w_d.rearrange("(a p) j -> p a j", p=128))
        wk_sb, wv_sb = w_sbs["k"], w_sbs["v"]

        for tt in range(N_TT):
            x_t = xp.tile([128, D_MODEL], f32, tag="x")
            nc.sync.dma_start(out=x_t, in_=x_d[tt * 128:(tt + 1) * 128, :])
            stats = sp.tile([128, 2, 6], f32, tag="bn")
            nc.vector.bn_stats(out=stats[:, 0, :], in_=x_t[:, 0:512])
            nc.vector.bn_stats(out=stats[:, 1, :], in_=x_t[:, 512:1024])
            mv = sp.tile([128, 2], f32, tag="mv")
            nc.vector.bn_aggr(out=mv, in_=stats)
            rstd = sp.tile([128, 1], f32, tag="rstd")
            nc.scalar.activation(out=rstd, in_=mv[:, 1:2], func=AF.Sqrt,
                                 bias=eps_sb, scale=1.0)
            nc.vector.reciprocal(out=rstd, in_=rstd)
            nmr = sp.tile([128, 1], f32, tag="nmr")
            nc.vector.tensor_scalar(out=nmr, in0=mv[:, 0:1], scalar1=rstd,
                                    scalar2=-1.0, op0=ALU.mult, op1=ALU.mult)
            xs_t = xp.tile([128, D_MODEL], bf16, tag="xs")
            nc.scalar.activation(out=xs_t, in_=x_t, func=AF.Identity,
                                 scale=rstd, bias=nmr)
            nc.sync.dma_start_transpose(
                out=xT[:, :, tt * 128:(tt + 1) * 128], in_=xs_t)

            # v projection for this tile: out[t, j] + bias rider matmul
            ps = psA.tile([128, 512], f32, tag="pv")
            for d in range(N_DT):
                nc.tensor.matmul(
                    ps,
                    xT[:, d, tt * 128:(tt + 1) * 128],
                    wv_sb[:, d, :],
                    start=(d == 0), stop=False,
                )
            nc.tensor.matmul(ps, ones_row, bv_row, start=False, stop=True)
            psv = ps.rearrange("p (h c) -> p h c", c=64)
            nc.scalar.copy(
                out=va_pair[:, tt, :, 0:64],
                in_=psv[:, 0::2, :])
            nc.scalar.copy(
                out=va_pair[:, tt, :, PB_O_OFF + 64:PB_O_OFF + 128],
                in_=psv[:, 1::2, :])

            if tt % 4 == 3:  # 512-token chunk boundary: run k
                tc_i = tt // 4
                cs = slice(tc_i * 512, (tc_i + 1) * 512)
                for jt in range(4):
                    ps = psA.tile([128, 512], f32, tag="pj")
                    for d in range(N_DT):
                        nc.tensor.matmul(
                            ps,
                            wk_sb[:, d, jt * 128:(jt + 1) * 128],
                            xT[:, d, cs],
                            start=(d == 0), stop=(d == N_DT - 1),
                        )
                    nc.scalar.activation(
                        out=kT[:, jt, cs], in_=ps,
                        func=AF.Identity, bias=bk_sb[:, jt:jt + 1], scale=1.0)
                if tc_i == 0:
                    # q chunk 0 right behind k chunk 0 (needed by phase B
                    # period 0; chunks 1..3 are emitted inside phase B)
                    for jt in range(4):
                        ps = psA.tile([128, 512], f32, tag="pj")
                        for d in range(N_DT):
                            nc.tensor.matmul(
                                ps,
                                wq_sb[:, d, jt * 128:(jt + 1) * 128],
                                xT[:, d, 0:512],
                                start=(d == 0), stop=(d == N_DT - 1),
                            )
                        nc.scalar.activation(
                            out=qT[:, jt, 0:512], in_=ps,
                            func=AF.Identity, bias=bq_sb[:, jt:jt + 1],
                            scale=1.0)


def _phase_bc(tc, nc, mybir, qT, kT, xT, wq_sb, bq_sb, v_aug, va_pair,
              attn_sb, wo_d, out_d, r_d, mu_val, recip_act2,
              RECIPROCAL_APPROX_FAST, rc):
    """Attention (phase B) with interleaved output projection (phase C)."""
    f32 = mybir.dt.float32
    bf16 = mybir.dt.bfloat16
    AF = mybir.ActivationFunctionType
    ALU = mybir.AluOpType

    with (
        tc.tile_pool(name="wo", bufs=1) as wop,
        tc.tile_pool(name="gb", bufs=2) as gp,
        tc.tile_pool(name="yb", bufs=3) as yp,
        tc.tile_pool(name="nrm", bufs=2) as nrm,
        tc.tile_pool(name="ob", bufs=2) as op_,
        tc.tile_pool(name="psS", bufs=2, space="PSUM") as psS,
        tc.tile_pool(name="psV", bufs=2, space="PSUM") as psV,
    ):
        wo_sb = wop.tile([128, 4, D_MODEL], bf16)
        nc.sync.dma_start(out=wo_sb, in_=wo_d.rearrange("(a p) e -> p a e", p=128))

        blocks = [(t1b, hp) for t1b in range(N_T1B) for hp in range(N_HP)]
        n_blk = len(blocks)
        # per-block state carried across the pipeline
        st = [None] * n_blk  # dict: g, av_e, av_o, den_e, den_o, r, bc

        def emit_scores_tile(i, t2t):
            t1b, hp = blocks[i]
            t1s = slice(t1b * 512, (t1b + 1) * 512)
            ps = psS.tile([128, 1024], f32, tag="sc")
            for par in range(2):
                rows = slice(par * 64, par * 64 + 64)
                nc.tensor.matmul(
                    ps[:, par * 512:(par + 1) * 512],
                    kT[rows, hp, t2t * 128:(t2t + 1) * 128],
                    qT[rows, hp, t1s],
                    start=True, stop=True,
                )
            y_t = yp.tile([128, 1024], bf16, tag="y")
            nc.scalar.activation(out=y_t, in_=ps, func=AF.Exp,
                                 scale=-1.0 / SCALE, bias=mu_val)
            nc.vector._custom_dve(
                recip_act2, out=st[i]["g"][:, t2t, :], in0=y_t,
                s0=RA_C0, s1=RA_C1)

        def emit_av_tile(i, t2t):
            t1b, hp = blocks[i]
            s = st[i]
            nc.tensor.matmul(
                s["av_e"],
                va_pair[:, t2t, hp, PB_E_OFF:PB_E_OFF + 65],
                s["g"][:, t2t, 0:512],
                start=(t2t == 0), stop=(t2t == N_TT - 1),
            )
            nc.tensor.matmul(
                s["av_o"],
                va_pair[:, t2t, hp, PB_O_OFF:PB_O_OFF + 128],
                s["g"][:, t2t, 512:1024],
                start=(t2t == 0), stop=(t2t == N_TT - 1),
            )

        def emit_den_extract(i):
            """Scalar-copy den rows from PSUM, DMA both to partition 0."""
            s = st[i]
            den_e = nrm.tile([65, 512], bf16, tag="den_e", bufs=1)
            nc.scalar.copy(out=den_e[64:65, :], in_=s["av_e"][64:65, :])
            den_o = nrm.tile([33, 512], bf16, tag="den_o", bufs=1)
            nc.scalar.copy(out=den_o[32:33, :], in_=s["av_o"][32:33, :])
            den_p0 = nrm.tile([1, 1024], bf16, tag="den_p0", bufs=1)
            nc.sync.dma_start(out=den_p0[0:1, 0:512], in_=den_e[64:65, :])
            nc.sync.dma_start(out=den_p0[0:1, 512:1024], in_=den_o[32:33, :])
            s["den_p0"] = den_p0

        def emit_recip(i):
            s = st[i]
            r_t = nrm.tile([1, 1024], bf16, tag="r", bufs=1)
            nc.vector._custom_dve(
                RECIPROCAL_APPROX_FAST, out=r_t, in0=s["den_p0"],
                s0=rc["s0"], s1=rc["s1"], imm2=rc["imm2"])
            s["r"] = r_t

        def emit_bcast(i):
            s = st[i]
            nc.sync.dma_start(out=r_d[i:i + 1, :], in_=s["r"])
            bc = nrm.tile([128, 512], bf16, tag="bc")
            nc.sync.dma_start(out=bc[0:64, :],
                              in_=_bcast_rows(r_d[i:i + 1, 0:512], 64))
            nc.sync.dma_start(out=bc[64:128, :],
                              in_=_bcast_rows(r_d[i:i + 1, 512:1024], 64))
            s["bc"] = bc

        def emit_q_chunk(tc_i):
            cs = slice(tc_i * 512, (tc_i + 1) * 512)
            for jt in range(4):
                ps = psS.tile([128, 1024], f32, tag="sc")
                for d in range(N_DT):
                    nc.tensor.matmul(
                        ps[:, 0:512],
                        wq_sb[:, d, jt * 128:(jt + 1) * 128],
                        xT[:, d, cs],
                        start=(d == 0), stop=(d == N_DT - 1),
                    )
                nc.scalar.activation(
                    out=qT[:, jt, cs], in_=ps[:, 0:512],
                    func=AF.Identity, bias=bq_sb[:, jt:jt + 1], scale=1.0)

        def emit_norm_mults(i):
            t1b, hp = blocks[i]
            t1s = slice(t1b * 512, (t1b + 1) * 512)
            s = st[i]
            nc.vector.tensor_tensor(
                out=attn_sb[0:64, hp, t1s], in0=s["av_e"][0:64, :],
                in1=s["bc"][0:64, :], op=ALU.mult)
            nc.vector.tensor_tensor(
                out=attn_sb[64:128, hp, t1s], in0=s["av_o"][64:128, :],
                in1=s["bc"][64:128, :], op=ALU.mult)

        def emit_phase_c(t1b):
            for tt_loc in range(4):
                tt2 = t1b * 4 + tt_loc
                for et in range(2):
                    es = slice(et * 512, (et + 1) * 512)
                    ps = psS.tile([128, 1024], f32, tag="sc")
                    for dt in range(4):
                        nc.tensor.matmul(
                            ps[:, 0:512],
                            attn_sb[:, dt, tt2 * 128:(tt2 + 1) * 128],
                            wo_sb[:, dt, es],
                            start=(dt == 0), stop=(dt == 3),
                        )
                    o_t = op_.tile([128, 512], f32, tag="o")
                    nc.scalar.copy(out=o_t, in_=ps[:, 0:512])
                    nc.sync.dma_start(
                        out=out_d[tt2 * 128:(tt2 + 1) * 128, es], in_=o_t)

        # ---- pipelined emission ----
        for i in range(n_blk):
            st[i] = {
                "g": gp.tile([128, N_TT, 1024], bf16, tag="g", name=f"g{i}"),
                "av_e": psV.tile([65, 512], f32, tag="av_e", name=f"av_e{i}"),
                "av_o": psV.tile([128, 512], f32, tag="av_o", name=f"av_o{i}"),
            }
            for t2t in range(N_TT):
                # av first: its inputs are ready, so the PE never idles at
                # the step boundary while scores wait on the psS ring
                if i > 0:
                    emit_av_tile(i - 1, t2t)
                    if t2t == N_TT - 1:
                        # before the last exp so the den copies jump the
                        # Scalar queue and the DMA chain starts early
                        emit_den_extract(i - 1)
                emit_scores_tile(i, t2t)
                if i > 1:
                    # norm tail of block i-2, spread across the period so
                    # each stage's DMA latency is hidden before its consumer
                    if t2t == 1:
                        emit_recip(i - 2)
                    elif t2t == 2:
                        emit_bcast(i - 2)
                    elif t2t == 9:
                        emit_norm_mults(i - 2)
            # q projections for t1 chunks 1..3, interleaved into early periods
            if i < 3:
                emit_q_chunk(i + 1)
            # phase C for a finished 512-token t1 block: blocks of t1b T are
            # 4T..4T+3; their norms complete during period 4T+5 (i-2 lag).
            if i >= 5 and (i - 5) % 4 == 0:
                emit_phase_c((i - 5) // 4)

        # drain tail
        for t2t in range(N_TT):
            emit_av_tile(n_blk - 1, t2t)
        emit_den_extract(n_blk - 1)
        emit_recip(n_blk - 2)
        emit_bcast(n_blk - 2)
        emit_norm_mults(n_blk - 2)
        emit_recip(n_blk - 1)
        emit_bcast(n_blk - 1)
        emit_norm_mults(n_blk - 1)
        emit_phase_c(3)


_PROGRAM_CACHE = {}


def _get_program(mu_val: float):
    key = round(float(mu_val), 9)
    if key not in _PROGRAM_CACHE:
        _PROGRAM_CACHE[key] = _build_program(float(mu_val))
    return _PROGRAM_CACHE[key]


def make_core_inputs(sequence, ln_gamma, ln_beta, Wq, bq, Wk, bk, Wv, bv, Wo, bo, mu):
    """Host-side shard prep: per-core input dicts (gamma/beta folded into W/b)."""
    f = np.float32
    bf = _ml.bfloat16
    seq = np.asarray(sequence, f)
    g = np.asarray(ln_gamma, f)
    be = np.asarray(ln_beta, f)
    in_maps = []
    for c in range(N_CORES):
        b, grp = c // 2, c % 2
        blk = slice(W_LOC * grp, W_LOC * (grp + 1))
        Wqb = np.asarray(Wq, f)[blk]
        Wkb = np.asarray(Wk, f)[blk]
        Wvb = np.asarray(Wv, f)[blk]
        m = {
            "x": np.ascontiguousarray(seq[b]),
            "wqT": np.ascontiguousarray((Wqb * g[None, :]).T).astype(bf),
            "wkT": np.ascontiguousarray((Wkb * g[None, :]).T).astype(bf),
            "wvT": np.ascontiguousarray((Wvb * g[None, :]).T).astype(bf),
            "woT": np.ascontiguousarray(np.asarray(Wo, f)[:, blk].T).astype(bf),
            "bq": np.ascontiguousarray(np.asarray(bq, f)[blk] + Wqb @ be),
            "bk": np.ascontiguousarray(np.asarray(bk, f)[blk] + Wkb @ be),
            "bv": np.ascontiguousarray(
                (np.asarray(bv, f)[blk] + Wvb @ be).reshape(1, W_LOC)).astype(bf),
        }
        in_maps.append(m)
    return in_maps


def combine_outputs(results, bo):
    out = np.zeros((BATCH, SEQ, D_MODEL), np.float32)
    for c in range(N_CORES):
        out[c // 2] += results[c]["out"]
    out += np.asarray(bo, np.float32)[None, None, :]
    return out


def kernel(sequence, ln_gamma, ln_beta, Wq, bq, Wk, bk, Wv, bv, Wo, bo, mu,
           _trace=False):
    from concourse.bass_utils import run_bass_kernel_spmd

    mu_val = float(np.asarray(mu).reshape(-1)[0])
    nc = _get_program(mu_val)
    in_maps = make_core_inputs(sequence, ln_gamma, ln_beta, Wq, bq, Wk, bk,
                               Wv, bv, Wo, bo, mu)
    res = run_bass_kernel_spmd(nc, in_maps, list(range(N_CORES)), trace=_trace)
    out = combine_outputs(res.results, bo)
    if _trace:
        kernel.last_results = res
    return out
